# revision 2
# baseline (speedup 1.0000x reference)
"""Trainium2 Bass kernel for nn_Cache_68135361184561 (retrieval_knn).

Computation (per batch element b, bsz=8):
    q_b   = query[0, :, b, :]                  # [L=64, h=1024]
    k_b   = keys[:, b, :].reshape(128, 64, 1024)   # [N, L, h]
    att[b, n] = max_{i,j} q_b[i] . k_b[n, j]   # [128]
    topk_idx  = top-8 blocks by att

values (512 MB) is unused by the reference computation.

Sharding: batch b -> NeuronCore b (8 cores, fully batch-parallel).

Device kernel (per core):
  - stream keys as 64 natural tiles [128 j', 1024 h] (contiguous HBM reads)
  - PE-transpose 128x128 chunks (exact fp32) into PSUM
  - DVE-evict PSUM -> SBUF, rounding to float32r (required by the PE for
    full-rate 4-byte matmul)
  - score: S[i, j'] accumulated over 8 h-chunks of matmul
    (lhsT = qT chunk [128h, 64i] f32r, rhs = K^T [128h, 512j'] f32r)
  - DVE max-reduce S over (j within n-block), accumulate [64 i, 128 n]
  - final PE transpose + DVE max over i -> att [128 n]

Host: gathers per-core att, re-ranks top-16 candidate blocks in fp64 from
the raw inputs to produce exact top-8 indices.
"""
from contextlib import ExitStack

import numpy as np

TOPK = 8
TOPC = 16  # candidate blocks re-ranked on host in fp64
N_CORES = 8
L = 64
H = 1024
N_BLOCKS = 128
DK = L * H  # 65536
NT = 64  # natural keys tiles per core, each [128, 1024]
GROUPS = 16  # scoring groups; each group = 4 natural tiles = 512 j'

_CACHE = {}


def _build():
    import concourse.bacc as bacc
    import concourse.tile as tile
    import concourse.mybir as mybir
    from concourse import masks

    F32 = mybir.dt.float32
    F32R = mybir.dt.float32r

    nc = bacc.Bacc("TRN2", target_bir_lowering=False, debug=False)
    qT = nc.dram_tensor("qT", [H, L], F32, kind="ExternalInput").ap()
    keys = nc.dram_tensor("keys", [N_BLOCKS, DK], F32, kind="ExternalInput").ap()
    att = nc.dram_tensor("att", [N_BLOCKS, 1], F32, kind="ExternalOutput").ap()

    with tile.TileContext(nc) as tc, ExitStack() as ctx:
        kn_pool = ctx.enter_context(tc.tile_pool(name="kn", bufs=10))
        kt_pool = ctx.enter_context(tc.tile_pool(name="kt", bufs=4))
        small = ctx.enter_context(tc.tile_pool(name="small", bufs=1))
        pt_pool = ctx.enter_context(tc.tile_pool(name="pt", bufs=3, space="PSUM"))
        ps_pool = ctx.enter_context(tc.tile_pool(name="ps", bufs=2, space="PSUM"))

        ident = small.tile([128, 128], F32, tag="ident")
        masks.make_identity(nc, ident[:])

        # query: load [1024, 64] as [128, 8, 64] then round to f32r
        qtmp = small.tile([128, 8 * L], F32, tag="qtmp")
        nc.sync.dma_start(
            qtmp[:].rearrange("p (c i) -> p c i", i=L),
            qT.rearrange("(c p) i -> p c i", p=128),
        )
        qTr = small.tile([128, 8 * L], F32R, tag="qTr")
        nc.vector.tensor_copy(qTr[:], qtmp[:])

        # per-i accumulated block maxima [64 i, 128 n]
        att_acc = small.tile([L, N_BLOCKS], F32, tag="att_acc")

        for g in range(GROUPS):
            # 1) DMA 4 natural tiles: [128 part = (2 n-blocks x 64 j), 1024 h]
            kns = []
            for k in range(4):
                t = 4 * g + k
                kn = kn_pool.tile([128, H], F32, tag="kn")
                nc.sync.dma_start(
                    kn[:],
                    keys[2 * t : 2 * t + 2, :].rearrange(
                        "n (j h) -> (n j) h", h=H
                    ),
                )
                kns.append(kn)

            # 2) per h-chunk: transpose 4x [128,128] -> one PSUM bank,
            #    evict (round to f32r), score into S accumulator
            s_ps = ps_pool.tile([L, 512], F32, tag="s")
            for c in range(8):
                pt = pt_pool.tile([128, 512], F32, tag="pt")
                for k in range(4):
                    nc.tensor.matmul(
                        pt[:, 128 * k : 128 * (k + 1)],
                        kns[k][:, 128 * c : 128 * (c + 1)],
                        ident[:],
                        is_transpose=True,
                    )
                kt = kt_pool.tile([128, 512], F32R, tag="kt")
                nc.vector.tensor_copy(kt[:], pt[:])
                nc.tensor.matmul(
                    s_ps[:],
                    qTr[:, L * c : L * (c + 1)],
                    kt[:],
                    start=(c == 0),
                    stop=(c == 7),
                )

            # 3) reduce max over j within each of the 8 n-blocks
            nc.vector.reduce_max(
                att_acc[:, 8 * g : 8 * (g + 1)],
                s_ps[:].rearrange("i (n j) -> i n j", j=L),
                axis=mybir.AxisListType.X,
            )

        # final: transpose [64 i, 128 n] -> [128 n, 64 i], max over i
        pfin = ps_pool.tile([128, L], F32, tag="pfin")
        nc.tensor.matmul(
            pfin[:], att_acc[:], ident[0:L, 0:L], is_transpose=True
        )
        fin = small.tile([N_BLOCKS, 1], F32, tag="fin")
        nc.vector.reduce_max(fin[:], pfin[:], axis=mybir.AxisListType.X)
        nc.sync.dma_start(att, fin[:])

    nc.compile()
    return nc


def _get_nc():
    if "nc" not in _CACHE:
        _CACHE["nc"] = _build()
    return _CACHE["nc"]


def kernel(query: np.ndarray, keys: np.ndarray, values: np.ndarray):
    from concourse import bass_utils

    assert query.shape == (1, L, N_CORES, H)
    assert keys.shape == (N_BLOCKS, N_CORES, DK)

    nc = _get_nc()

    in_maps = []
    for b in range(N_CORES):
        qb = np.ascontiguousarray(query[0, :, b, :].T, dtype=np.float32)  # [H, L]
        kb = np.ascontiguousarray(keys[:, b, :], dtype=np.float32)  # [N, DK]
        in_maps.append({"qT": qb, "keys": kb})

    res = bass_utils.run_bass_kernel_spmd(
        nc, in_maps, core_ids=list(range(N_CORES)), **_CACHE.get("run_kwargs", {})
    )
    _CACHE["last_result"] = res

    att = np.empty((N_CORES, 1, N_BLOCKS), dtype=np.float32)
    for b in range(N_CORES):
        att[b, 0, :] = res.results[b]["att"][:, 0]

    # exact top-k: re-rank top candidate blocks in fp64 from raw inputs
    topk = np.empty((TOPK, N_CORES), dtype=np.int32)
    for b in range(N_CORES):
        cand = np.argsort(-att[b, 0], kind="stable")[:TOPC]
        qb = query[0, :, b, :].astype(np.float64)  # [64, 1024]
        kb = keys[cand, b, :].reshape(TOPC, L, H).astype(np.float64)
        # scores[n] = max_{i,j} q[i] . k[n, j]
        s = np.einsum("ih,njh->nij", qb, kb, optimize=True)
        sc = s.reshape(TOPC, -1).max(axis=1)
        order = np.argsort(-sc, kind="stable")[:TOPK]
        topk[:, b] = cand[order].astype(np.int32)

    return att, topk
